# revision 42
# baseline (speedup 1.0000x reference)
"""Multi-head attention (nn_MultiHeadAttention_84052509983487) on 8 trn2 NeuronCores.

Sharding: core c handles batch b = c//2 and head-group hg = c%2 (4 of the 8 heads).
Each core computes its 4 heads' attention probs (stored transposed [k,q]) plus the
partial hidden projection; host sums the two half-head partials per batch.

Scores are computed transposed (scoresT[k,q]) so the softmax denominator comes out
of the ctx matmul for free (ones column appended to V) and the ctx contraction can
run without transposing the 16 MiB probability matrix.
"""

import numpy as np
import ml_dtypes

B, S, H, NH = 4, 2048, 512, 8
HD = H // NH          # 64
HPC = 4               # heads per core
NCORES = 8
P = 128               # partitions
KT = S // P           # 16 k-tiles
QCS = 512             # q-chunk size
NQC = S // QCS        # 4 q-chunks
CC = H // P           # 4 contraction chunks for projections
MASK_BIAS = -240.0    # exp(0.125 * -240) = exp(-30) ~ 9e-14  -> masked probs ~ 0

BF16 = ml_dtypes.bfloat16
FP8 = ml_dtypes.float8_e4m3

_cached = {}


def _split_multi_waits(nc):
    """Toolchain workaround: the walrus build in this container supports only ONE
    sync-wait per instruction, but Tile attaches several sem waits to a single
    instruction (the exit drain, and compute insts that depend on multiple
    producers).  Insert same-engine NoOp instructions, each carrying one of the
    extra waits, directly before the offending instruction — semantically
    identical because the engine's queue executes in program order."""
    import bass_rust
    from concourse import mybir

    n_split = 0
    for f in nc.m.functions:
        for bb in f.blocks:
            new = []
            changed = False
            for inst in bb.instructions:
                si = inst.sync_info
                if si is not None and si.on_wait and len(si.on_wait) > 1:
                    waits = list(si.on_wait)
                    for w in waits[:-1]:
                        n_split += 1
                        nop = mybir.InstEventSemaphore(
                            name=f"I-waitsplit-{n_split}",
                            engine=inst.engine,
                            ins=[],
                            outs=[],
                        )
                        nop.sync_info = bass_rust.SyncInfo(on_wait=[w], on_update=[])
                        new.append(nop)
                    si.on_wait = [waits[-1]]
                    inst.sync_info = si
                    changed = True
                new.append(inst)
            if changed:
                bb.instructions = new
    return n_split


def _build_program():
    """Build the SPMD Bass program (identical for all 8 cores; data differs)."""
    from contextlib import ExitStack

    import concourse.bass as bass
    import concourse.tile as tile
    from concourse import mybir

    f32 = mybir.dt.float32
    bf16 = mybir.dt.bfloat16
    AF = mybir.ActivationFunctionType

    nc = bass.Bass("TRN2", target_bir_lowering=False, debug=False, num_devices=1)

    ein = dict(kind="ExternalInput")
    qT = nc.dram_tensor("qT", [H, S], bf16, **ein).ap()          # [c, s]
    kTT = nc.dram_tensor("kTT", [H, S], bf16, **ein).ap()        # [c, s]
    vT = nc.dram_tensor("vT", [H, S], bf16, **ein).ap()          # [c, s]
    fp8 = mybir.dt.float8e4
    maskT = nc.dram_tensor("maskT", [S, S], fp8, **ein).ap()    # [k, q] = -240*mask
    WqTd = nc.dram_tensor("WqTd", [H, HPC * HD], bf16, **ein).ap()   # [c, d4]
    WvTd = nc.dram_tensor("WvTd", [H, HPC * HD], bf16, **ein).ap()   # [c, d4]
    WdTd = nc.dram_tensor("WdTd", [HPC * HD, H], bf16, **ein).ap()   # [d4, o]
    bq4 = nc.dram_tensor("bq4", [HPC, HD, 1], f32, **ein).ap()
    bv4 = nc.dram_tensor("bv4", [1, HPC, HD], bf16, **ein).ap()
    iden = nc.dram_tensor("iden", [P, P], fp8, **ein).ap()
    onesf = nc.dram_tensor("onesf", [1, P], f32, **ein).ap()
    ones1 = nc.dram_tensor("ones1", [1, P], bf16, **ein).ap()

    probs = nc.dram_tensor("probs", [HPC, S, S], f32, kind="ExternalOutput").ap()
    hid = nc.dram_tensor("hid", [S, H], f32, kind="ExternalOutput").ap()

    with tile.TileContext(nc) as tc, ExitStack() as ctx:
        wpool = ctx.enter_context(tc.tile_pool(name="wpool", bufs=1))
        hpool = ctx.enter_context(tc.tile_pool(name="hpool", bufs=1))
        ps = ctx.enter_context(tc.tile_pool(name="ps", bufs=1, space="PSUM"))

        iden_sb = wpool.tile([P, P], fp8, name="iden_sb")
        nc.scalar.dma_start(out=iden_sb, in_=iden)
        onesf_sb = wpool.tile([1, P], f32, name="onesf_sb")
        nc.scalar.dma_start(out=onesf_sb, in_=onesf)
        ones1_sb = wpool.tile([1, P], bf16, name="ones1_sb")
        nc.scalar.dma_start(out=ones1_sb, in_=ones1)
        WdT_sb = wpool.tile([HD, HPC, H], bf16, name="WdT_sb")
        nc.scalar.dma_start(out=WdT_sb, in_=WdTd.rearrange("(h d) o -> d h o", d=HD))
        # bias packed to match the head-pair layout: bqp2[p, pi] = bq[pi*128+p]
        bqp_sb = wpool.tile([P, HPC // 2], f32, name="bqp_sb")
        nc.scalar.dma_start(
            out=bqp_sb,
            in_=bq4.rearrange("(pi hp) d o -> (hp d) (pi o)", pi=HPC // 2, hp=2),
        )
        bv_sb = wpool.tile([1, HPC, HD], bf16, name="bv_sb")
        nc.scalar.dma_start(out=bv_sb, in_=bv4)

        # Head-PAIR packed projections: pack pi holds heads 2pi (rows 0:64)
        # and 2pi+1 (rows 64:128).  Odd heads' score matmuls then run on PE
        # row-groups 2-3 while even heads use 0-1, which the PE overlaps.
        qpk, kpk = [], []
        for pi in range(HPC // 2):
            t = hpool.tile([P, S], bf16, name=f"qpk{pi}", tag=f"qpk{pi}")
            qpk.append(t)
            t = hpool.tile([P, S], bf16, name=f"kpk{pi}", tag=f"kpk{pi}")
            kpk.append(t)

        def qh_view(hh):
            return qpk[hh // 2][bass.ds((hh % 2) * HD, HD), :]

        def kh_view(hh):
            return kpk[hh // 2][bass.ds((hh % 2) * HD, HD), :]

        vaug = hpool.tile([P, KT, HPC, HD + 1], bf16, name="vaug")

        # mask chunks: pool + loader declared early so the first two chunks'
        # loads sit at the head of the ACT sequencer stream (not queued behind
        # a chunk's worth of exp instructions)
        maskp = ctx.enter_context(tc.tile_pool(name="maskp", bufs=2))
        mask_tiles = {}

        def load_mask(mqc):
            if mqc in mask_tiles or mqc >= NQC:
                return
            msl = bass.ds(mqc * QCS, QCS)
            m = maskp.tile([P, KT, QCS], fp8, name=f"mask{mqc}", tag="mask")
            nc.scalar.dma_start(
                out=m, in_=maskT[:, msl].rearrange("(kt p) q -> p kt q", p=P)
            )
            mask_tiles[mqc] = m

        # ---------------- projections ----------------
        with tc.tile_pool(name="io", bufs=1) as io:
            # chunked q/k loads so the first projection matmul starts early
            # load order = first-use order: WqT, then q chunks (head-0 pack
            # projection starts while k still streams), then k, then v
            WqT_sb = io.tile([P, CC, HPC * HD], bf16, name="WqT_sb")
            nc.scalar.dma_start(
                out=WqT_sb, in_=WqTd.rearrange("(cc p) d -> p cc d", p=P)
            )
            qT_sb = io.tile([P, CC, S], bf16, name="qT_sb")
            kT_sb = io.tile([P, CC, S], bf16, name="kT_sb")
            qTr = qT.rearrange("(cc p) s -> p cc s", p=P)
            kTr = kTT.rearrange("(cc p) s -> p cc s", p=P)
            for cc in range(CC):
                nc.scalar.dma_start(out=qT_sb[:, cc, :], in_=qTr[:, cc, :])
            for cc in range(CC):
                nc.scalar.dma_start(out=kT_sb[:, cc, :], in_=kTr[:, cc, :])
            WvT_sb = io.tile([P, CC, HPC * HD], bf16, name="WvT_sb")
            nc.scalar.dma_start(
                out=WvT_sb, in_=WvTd.rearrange("(cc p) d -> p cc d", p=P)
            )
            vT_sb = io.tile([P, CC, S], bf16, name="vT_sb")
            nc.scalar.dma_start(out=vT_sb, in_=vT.rearrange("(cc p) s -> p cc s", p=P))
            load_mask(0)
            load_mask(1)

            # q/k pair projections: out [2 heads' d (128), s] per pack, so the
            # stationary operand uses the full 128-wide PE array.
            # (matmul PSUM output is limited to one bank = 512 fp32 columns)
            # single-bank ("aux"-tag) PSUM tiles so projections never compete
            # with the score matmuls for the "sc" slots
            for pi in range(HPC // 2):
                dsl = bass.ds(pi * P, P)
                for src_sb, dst, tagn in (
                    (qT_sb, qpk[pi], "pq"),
                    (kT_sb, kpk[pi], "pk"),
                ):
                    for quarter in range(S // 512):
                        qoff = quarter * 512
                        pps = ps.tile(
                            [P, 512],
                            f32,
                            tag="aux",
                            bufs=2,
                            name=f"pp_{tagn}{pi}_{quarter}",
                        )
                        for cc in range(CC):
                            nc.tensor.matmul(
                                pps,
                                lhsT=WqT_sb[:, cc, dsl],
                                rhs=src_sb[:, cc, bass.ds(qoff, 512)],
                                start=(cc == 0),
                                stop=(cc == CC - 1),
                            )
                        # evac + bias (k uses bq too: faithful to the source bug)
                        nc.scalar.activation(
                            out=dst[:, bass.ds(qoff, 512)],
                            in_=pps,
                            func=AF.Identity,
                            bias=bqp_sb[:, pi : pi + 1],
                            scale=1.0,
                        )

            # v projection: out v[k, d] = sum_c vT[c, k] * WvT[c, d]  (+ bv)
            for kt in range(KT):
                vps = ps.tile([P, HPC * HD], f32, tag="aux", bufs=2, name=f"vps{kt}")
                for cc in range(CC):
                    for hh in range(HPC):
                        # start=True clears has_written for the WHOLE bank, so
                        # only the very first matmul into this tile may set it.
                        nc.tensor.matmul(
                            vps[:, bass.ds(hh * HD, HD)],
                            lhsT=vT_sb[:, cc, bass.ds(kt * P, P)],
                            rhs=WvT_sb[:, cc, bass.ds(hh * HD, HD)],
                            start=(cc == 0 and hh == 0),
                            stop=False,
                            skip_group_check=True,
                        )
                for hh in range(HPC):
                    nc.tensor.matmul(
                        vps[:, bass.ds(hh * HD, HD)],
                        lhsT=ones1_sb,
                        rhs=bv_sb[:, hh, :],
                        start=False,
                        stop=(hh == HPC - 1),
                        skip_group_check=True,
                    )
                nc.scalar.activation(
                    out=vaug[:, kt, :, 0:HD], in_=vps, func=AF.Copy
                )
            nc.vector.memset(vaug[:, :, :, HD : HD + 1], 1.0)

        # ---------------- main attention loop ----------------
        pbp = ctx.enter_context(tc.tile_pool(name="pbp", bufs=2))
        pfp = ctx.enter_context(tc.tile_pool(name="pfp", bufs=2))
        smp = ctx.enter_context(tc.tile_pool(name="smp", bufs=2))

        GK = 2   # k-tiles per PSUM scores group
        PIECE = 4  # k-tiles per probs store

        for qc in range(NQC):
            qsl = bass.ds(qc * QCS, QCS)
            load_mask(qc)
            load_mask(qc + 1)
            mask_sb = mask_tiles[qc]
            ctxn_tiles = []
            for hh in range(HPC):
                pb = pbp.tile([P, KT, QCS], bf16, name=f"pb{qc}_{hh}", tag="pb", bufs=3)
                caug = ps.tile(
                    [HD + 1, QCS], f32, tag="ctx", bufs=2, name=f"caug{qc}_{hh}"
                )
                for g in range(KT // GK):
                    sc = ps.tile(
                        [P, GK * QCS], f32, tag="sc", bufs=2, name=f"sc{qc}_{hh}_{g}"
                    )
                    for j in range(GK):
                        kt = GK * g + j
                        nc.tensor.matmul(
                            sc[:, bass.ds(j * QCS, QCS)],
                            lhsT=kh_view(hh)[:, bass.ds(kt * P, P)],
                            rhs=qh_view(hh)[:, qsl],
                            start=True,
                            stop=False,
                        )
                    for j in range(GK):
                        kt = GK * g + j
                        nc.tensor.matmul(
                            sc[:, bass.ds(j * QCS, QCS)],
                            lhsT=iden_sb,
                            rhs=mask_sb[:, kt, :],
                            start=False,
                            stop=True,
                        )
                    nc.scalar.activation(
                        out=pb[:, bass.ds(GK * g, GK), :],
                        in_=sc,
                        func=AF.Exp,
                        scale=0.125,
                    )
                    for j in range(GK):
                        kt = GK * g + j
                        nc.tensor.matmul(
                            caug,
                            lhsT=vaug[:, kt, hh, :],
                            rhs=pb[:, kt, :],
                            start=(kt == 0),
                            stop=(kt == KT - 1),
                        )
                # softmax denominator -> 1/x as exp(-ln(x)) on ACT (the DVE
                # reciprocal ops hit a codegen bug in this toolchain, and
                # exp/ln/copy/identity share one ACT table set so no reload),
                # then broadcast to 128 partitions with a K=1 PE matmul.
                rs_sb = smp.tile([1, QCS], f32, name=f"rs{qc}_{hh}", tag="rs")
                nc.scalar.activation(out=rs_sb, in_=caug[HD : HD + 1, :], func=AF.Ln)
                recip = smp.tile([1, QCS], f32, name=f"rc{qc}_{hh}", tag="rc")
                nc.scalar.activation(out=recip, in_=rs_sb, func=AF.Exp, scale=-1.0)
                bps = ps.tile([P, QCS], f32, tag="aux", bufs=2, name=f"bps{qc}_{hh}")
                nc.tensor.matmul(bps, lhsT=onesf_sb, rhs=recip, start=True, stop=True)
                aux_sb = smp.tile([P, QCS], f32, name=f"aux{qc}_{hh}", tag="aux_sb")
                nc.scalar.activation(out=aux_sb, in_=bps, func=AF.Copy)

                # normalized ctx (bf16) first: it gates the hidden projection,
                # so don't queue it behind the 16 probs-normalize TTs on DVE
                ctxn = smp.tile(
                    [HD, QCS], bf16, name=f"ctxn{qc}_{hh}", tag=f"ctxn{hh}"
                )
                nc.vector.tensor_mul(ctxn, caug[0:HD, :], aux_sb[0:HD, :])
                ctxn_tiles.append(ctxn)

                # normalized probs -> fp32 -> DRAM (stored [k, q]), 4-kt pieces
                for pc in range(KT // PIECE):
                    pf = pfp.tile(
                        [P, PIECE, QCS], f32, name=f"pf{qc}_{hh}_{pc}", tag="pf",
                        bufs=4,
                    )
                    for j in range(PIECE):
                        kt = pc * PIECE + j
                        nc.vector.tensor_mul(pf[:, j, :], pb[:, kt, :], aux_sb)
                    nc.sync.dma_start(
                        out=probs[
                            hh, bass.ds(pc * PIECE * P, PIECE * P), qsl
                        ].rearrange("(kt p) q -> p kt q", p=P),
                        in_=pf,
                    )


            # partial hidden for this q-chunk: sum over this core's 4 heads
            hid_sb = smp.tile(
                [P, QCS // P, H], f32, name=f"hid{qc}", tag="hid", bufs=1
            )
            for st in range(QCS // P):
                hps = ps.tile([P, H], f32, tag="aux", bufs=2, name=f"hps{qc}_{st}")
                for hh in range(HPC):
                    nc.tensor.matmul(
                        hps,
                        lhsT=ctxn_tiles[hh][:, bass.ds(st * P, P)],
                        rhs=WdT_sb[:, hh, :],
                        start=(hh == 0),
                        stop=(hh == HPC - 1),
                    )
                nc.vector.tensor_copy(out=hid_sb[:, st, :], in_=hps)
            nc.sync.dma_start(
                out=hid[qsl, :].rearrange("(st p) o -> p st o", p=P),
                in_=hid_sb,
            )

    _split_multi_waits(nc)
    return nc


def _get_program():
    if "nc" not in _cached:
        _cached["nc"] = _build_program()
    return _cached["nc"]


def kernel(query, key, value, pad_mask, Wq, bq, Wv, bv, Wd, bd):
    from concourse import bass_utils

    query = np.asarray(query, dtype=np.float32)
    key = np.asarray(key, dtype=np.float32)
    value = np.asarray(value, dtype=np.float32)
    pad_mask = np.asarray(pad_mask)
    Wq = np.asarray(Wq, dtype=np.float32)
    bq = np.asarray(bq, dtype=np.float32)
    Wv = np.asarray(Wv, dtype=np.float32)
    bv = np.asarray(bv, dtype=np.float32)
    Wd = np.asarray(Wd, dtype=np.float32)
    bd = np.asarray(bd, dtype=np.float32)

    # host-side prep (transposes + bf16 casts + per-core weight slices)
    qT_all = np.ascontiguousarray(query.transpose(0, 2, 1)).astype(BF16)   # [B, c, s]
    kT_all = np.ascontiguousarray(key.transpose(0, 2, 1)).astype(BF16)
    vT_all = np.ascontiguousarray(value.transpose(0, 2, 1)).astype(BF16)
    maskT_all = np.ascontiguousarray(
        (pad_mask.transpose(0, 2, 1).astype(np.float32)) * MASK_BIAS
    ).astype(FP8)                                                           # [B, k, q]
    WqT = np.ascontiguousarray(Wq.T).astype(BF16)    # [c, d]
    WvT = np.ascontiguousarray(Wv.T).astype(BF16)
    WdT = np.ascontiguousarray(Wd.T).astype(BF16)    # [d, o]
    iden = np.eye(P, dtype=np.float32).astype(FP8)
    onesf = np.ones((1, P), np.float32)
    ones1 = np.ones((1, P), np.float32).astype(BF16)

    in_maps = []
    for c in range(NCORES):
        b, hg = c // 2, c % 2
        dsl = slice(hg * HPC * HD, (hg + 1) * HPC * HD)
        in_maps.append(
            {
                "qT": qT_all[b],
                "kTT": kT_all[b],
                "vT": vT_all[b],
                "maskT": maskT_all[b],
                "WqTd": np.ascontiguousarray(WqT[:, dsl]),
                "WvTd": np.ascontiguousarray(WvT[:, dsl]),
                "WdTd": np.ascontiguousarray(WdT[dsl, :]),
                "bq4": np.ascontiguousarray(
                    bq[dsl].reshape(HPC, HD, 1)
                ).astype(np.float32),
                "bv4": np.ascontiguousarray(
                    bv[dsl].reshape(1, HPC, HD)
                ).astype(BF16),
                "iden": iden,
                "onesf": onesf,
                "ones1": ones1,
            }
        )

    nc = _get_program()
    import os

    trace = bool(int(os.environ.get("MHA_TRACE", "0")))
    kw = {}
    if trace:
        kw = dict(trace=True, trace_cores=[0], tmpdir=os.environ.get("MHA_TRACE_DIR"))
    res = bass_utils.run_bass_kernel_spmd(
        nc, in_maps, core_ids=list(range(NCORES)), **kw
    )
    _cached["last_results"] = res

    # assemble outputs
    probs_kq = np.empty((B, 2, HPC, S, S), np.float32)
    hidden = np.empty((B, S, H), np.float32)
    for c in range(NCORES):
        b, hg = c // 2, c % 2
        probs_kq[b, hg] = res.results[c]["probs"]
        if hg == 0:
            hidden[b] = res.results[c]["hid"]
        else:
            hidden[b] += res.results[c]["hid"]
    hidden += bd[None, None, :]
    # [B, hg, hh, k, q] -> [B, NH, q, k] as a zero-copy view
    attn_probs = probs_kq.reshape(B, NH, S, S).transpose(0, 1, 3, 2)
    return hidden, attn_probs


# revision 45
# speedup vs baseline: 1.0053x; 1.0053x over previous
"""Multi-head attention (nn_MultiHeadAttention_84052509983487) on 8 trn2 NeuronCores.

Sharding: core c handles batch b = c//2 and head-group hg = c%2 (4 of the 8 heads).
Each core computes its 4 heads' attention probs (stored transposed [k,q]) plus the
partial hidden projection; host sums the two half-head partials per batch.

Scores are computed transposed (scoresT[k,q]) so the softmax denominator comes out
of the ctx matmul for free (ones column appended to V) and the ctx contraction can
run without transposing the 16 MiB probability matrix.
"""

import numpy as np
import ml_dtypes

B, S, H, NH = 4, 2048, 512, 8
HD = H // NH          # 64
HPC = 4               # heads per core
NCORES = 8
P = 128               # partitions
KT = S // P           # 16 k-tiles
QCS = 512             # q-chunk size
NQC = S // QCS        # 4 q-chunks
CC = H // P           # 4 contraction chunks for projections
MASK_BIAS = -240.0    # exp(0.125 * -240) = exp(-30) ~ 9e-14  -> masked probs ~ 0

BF16 = ml_dtypes.bfloat16
FP8 = ml_dtypes.float8_e4m3

_cached = {}


def _split_multi_waits(nc):
    """Toolchain workaround: the walrus build in this container supports only ONE
    sync-wait per instruction, but Tile attaches several sem waits to a single
    instruction (the exit drain, and compute insts that depend on multiple
    producers).  Insert same-engine NoOp instructions, each carrying one of the
    extra waits, directly before the offending instruction — semantically
    identical because the engine's queue executes in program order."""
    import bass_rust
    from concourse import mybir

    n_split = 0
    for f in nc.m.functions:
        for bb in f.blocks:
            new = []
            changed = False
            for inst in bb.instructions:
                si = inst.sync_info
                if si is not None and si.on_wait and len(si.on_wait) > 1:
                    waits = list(si.on_wait)
                    for w in waits[:-1]:
                        n_split += 1
                        nop = mybir.InstEventSemaphore(
                            name=f"I-waitsplit-{n_split}",
                            engine=inst.engine,
                            ins=[],
                            outs=[],
                        )
                        nop.sync_info = bass_rust.SyncInfo(on_wait=[w], on_update=[])
                        new.append(nop)
                    si.on_wait = [waits[-1]]
                    inst.sync_info = si
                    changed = True
                new.append(inst)
            if changed:
                bb.instructions = new
    return n_split


def _build_program():
    """Build the SPMD Bass program (identical for all 8 cores; data differs)."""
    from contextlib import ExitStack

    import concourse.bass as bass
    import concourse.tile as tile
    from concourse import mybir

    f32 = mybir.dt.float32
    bf16 = mybir.dt.bfloat16
    AF = mybir.ActivationFunctionType

    nc = bass.Bass("TRN2", target_bir_lowering=False, debug=False, num_devices=1)

    ein = dict(kind="ExternalInput")
    qT = nc.dram_tensor("qT", [H, S], bf16, **ein).ap()          # [c, s]
    kTT = nc.dram_tensor("kTT", [H, S], bf16, **ein).ap()        # [c, s]
    vT = nc.dram_tensor("vT", [H, S], bf16, **ein).ap()          # [c, s]
    fp8 = mybir.dt.float8e4
    maskT = nc.dram_tensor("maskT", [S, S], fp8, **ein).ap()    # [k, q] = -240*mask
    WqTd = nc.dram_tensor("WqTd", [H, HPC * HD], bf16, **ein).ap()   # [c, d4]
    WvTd = nc.dram_tensor("WvTd", [H, HPC * HD], bf16, **ein).ap()   # [c, d4]
    WdTd = nc.dram_tensor("WdTd", [HPC * HD, H], bf16, **ein).ap()   # [d4, o]
    bq4 = nc.dram_tensor("bq4", [HPC, HD, 1], f32, **ein).ap()
    bv4 = nc.dram_tensor("bv4", [1, HPC, HD], bf16, **ein).ap()
    iden = nc.dram_tensor("iden", [P, P], fp8, **ein).ap()
    onesf = nc.dram_tensor("onesf", [1, P], f32, **ein).ap()
    ones1 = nc.dram_tensor("ones1", [1, P], bf16, **ein).ap()

    probs = nc.dram_tensor("probs", [HPC, S, S], f32, kind="ExternalOutput").ap()
    hid = nc.dram_tensor("hid", [S, H], f32, kind="ExternalOutput").ap()

    with tile.TileContext(nc) as tc, ExitStack() as ctx:
        wpool = ctx.enter_context(tc.tile_pool(name="wpool", bufs=1))
        hpool = ctx.enter_context(tc.tile_pool(name="hpool", bufs=1))
        ps = ctx.enter_context(tc.tile_pool(name="ps", bufs=1, space="PSUM"))

        iden_sb = wpool.tile([P, P], fp8, name="iden_sb")
        nc.scalar.dma_start(out=iden_sb, in_=iden)
        onesf_sb = wpool.tile([1, P], f32, name="onesf_sb")
        nc.scalar.dma_start(out=onesf_sb, in_=onesf)
        ones1_sb = wpool.tile([1, P], bf16, name="ones1_sb")
        nc.scalar.dma_start(out=ones1_sb, in_=ones1)
        WdT_sb = wpool.tile([HD, HPC, H], bf16, name="WdT_sb")
        nc.scalar.dma_start(out=WdT_sb, in_=WdTd.rearrange("(h d) o -> d h o", d=HD))
        # bias packed to match the head-pair layout: bqp2[p, pi] = bq[pi*128+p]
        bqp_sb = wpool.tile([P, HPC // 2], f32, name="bqp_sb")
        nc.scalar.dma_start(
            out=bqp_sb,
            in_=bq4.rearrange("(pi hp) d o -> (hp d) (pi o)", pi=HPC // 2, hp=2),
        )
        bv_sb = wpool.tile([1, HPC, HD], bf16, name="bv_sb")
        nc.scalar.dma_start(out=bv_sb, in_=bv4)

        # Head-PAIR packed projections: pack pi holds heads 2pi (rows 0:64)
        # and 2pi+1 (rows 64:128).  Odd heads' score matmuls then run on PE
        # row-groups 2-3 while even heads use 0-1, which the PE overlaps.
        qpk, kpk = [], []
        for pi in range(HPC // 2):
            t = hpool.tile([P, S], bf16, name=f"qpk{pi}", tag=f"qpk{pi}")
            qpk.append(t)
            t = hpool.tile([P, S], bf16, name=f"kpk{pi}", tag=f"kpk{pi}")
            kpk.append(t)

        def qh_view(hh):
            return qpk[hh // 2][bass.ds((hh % 2) * HD, HD), :]

        def kh_view(hh):
            return kpk[hh // 2][bass.ds((hh % 2) * HD, HD), :]

        vaug = hpool.tile([P, KT, HPC, HD + 1], bf16, name="vaug")

        # mask chunks: pool + loader declared early so the first two chunks'
        # loads sit at the head of the ACT sequencer stream (not queued behind
        # a chunk's worth of exp instructions)
        maskp = ctx.enter_context(tc.tile_pool(name="maskp", bufs=2))
        mask_tiles = {}

        def load_mask(mqc):
            if mqc in mask_tiles or mqc >= NQC:
                return
            msl = bass.ds(mqc * QCS, QCS)
            m = maskp.tile([P, KT, QCS], fp8, name=f"mask{mqc}", tag="mask")
            nc.scalar.dma_start(
                out=m, in_=maskT[:, msl].rearrange("(kt p) q -> p kt q", p=P)
            )
            mask_tiles[mqc] = m

        # ---------------- projections ----------------
        with tc.tile_pool(name="io", bufs=1) as io:
            # chunked q/k loads so the first projection matmul starts early
            # load order = first-use order: WqT, then q chunks (head-0 pack
            # projection starts while k still streams), then k, then v
            WqT_sb = io.tile([P, CC, HPC * HD], bf16, name="WqT_sb")
            nc.scalar.dma_start(
                out=WqT_sb, in_=WqTd.rearrange("(cc p) d -> p cc d", p=P)
            )
            qT_sb = io.tile([P, CC, S], bf16, name="qT_sb")
            kT_sb = io.tile([P, CC, S], bf16, name="kT_sb")
            qTr = qT.rearrange("(cc p) s -> p cc s", p=P)
            kTr = kTT.rearrange("(cc p) s -> p cc s", p=P)
            for cc in range(CC):
                nc.scalar.dma_start(out=qT_sb[:, cc, :], in_=qTr[:, cc, :])
            for cc in range(CC):
                nc.scalar.dma_start(out=kT_sb[:, cc, :], in_=kTr[:, cc, :])
            WvT_sb = io.tile([P, CC, HPC * HD], bf16, name="WvT_sb")
            nc.scalar.dma_start(
                out=WvT_sb, in_=WvTd.rearrange("(cc p) d -> p cc d", p=P)
            )
            vT_sb = io.tile([P, CC, S], bf16, name="vT_sb")
            nc.scalar.dma_start(out=vT_sb, in_=vT.rearrange("(cc p) s -> p cc s", p=P))
            load_mask(0)
            load_mask(1)

            # q/k pair projections: out [2 heads' d (128), s] per pack, so the
            # stationary operand uses the full 128-wide PE array.
            # (matmul PSUM output is limited to one bank = 512 fp32 columns)
            # single-bank ("aux"-tag) PSUM tiles so projections never compete
            # with the score matmuls for the "sc" slots
            for pi in range(HPC // 2):
                dsl = bass.ds(pi * P, P)
                for src_sb, dst, tagn in (
                    (qT_sb, qpk[pi], "pq"),
                    (kT_sb, kpk[pi], "pk"),
                ):
                    for quarter in range(S // 512):
                        qoff = quarter * 512
                        pps = ps.tile(
                            [P, 512],
                            f32,
                            tag="aux",
                            bufs=2,
                            name=f"pp_{tagn}{pi}_{quarter}",
                        )
                        for cc in range(CC):
                            nc.tensor.matmul(
                                pps,
                                lhsT=WqT_sb[:, cc, dsl],
                                rhs=src_sb[:, cc, bass.ds(qoff, 512)],
                                start=(cc == 0),
                                stop=(cc == CC - 1),
                            )
                        # evac + bias (k uses bq too: faithful to the source bug)
                        nc.scalar.activation(
                            out=dst[:, bass.ds(qoff, 512)],
                            in_=pps,
                            func=AF.Identity,
                            bias=bqp_sb[:, pi : pi + 1],
                            scale=1.0,
                        )

            # v projection: out v[k, d] = sum_c vT[c, k] * WvT[c, d]  (+ bv)
            for kt in range(KT):
                vps = ps.tile([P, HPC * HD], f32, tag="aux", bufs=2, name=f"vps{kt}")
                for cc in range(CC):
                    for hh in range(HPC):
                        # start=True clears has_written for the WHOLE bank, so
                        # only the very first matmul into this tile may set it.
                        nc.tensor.matmul(
                            vps[:, bass.ds(hh * HD, HD)],
                            lhsT=vT_sb[:, cc, bass.ds(kt * P, P)],
                            rhs=WvT_sb[:, cc, bass.ds(hh * HD, HD)],
                            start=(cc == 0 and hh == 0),
                            stop=False,
                            skip_group_check=True,
                        )
                for hh in range(HPC):
                    nc.tensor.matmul(
                        vps[:, bass.ds(hh * HD, HD)],
                        lhsT=ones1_sb,
                        rhs=bv_sb[:, hh, :],
                        start=False,
                        stop=(hh == HPC - 1),
                        skip_group_check=True,
                    )
                nc.scalar.activation(
                    out=vaug[:, kt, :, 0:HD], in_=vps, func=AF.Copy
                )
            nc.vector.memset(vaug[:, :, :, HD : HD + 1], 1.0)

        # ---------------- main attention loop ----------------
        pbp = ctx.enter_context(tc.tile_pool(name="pbp", bufs=2))
        pfp = ctx.enter_context(tc.tile_pool(name="pfp", bufs=2))
        smp = ctx.enter_context(tc.tile_pool(name="smp", bufs=2))

        GK = 2   # k-tiles per PSUM scores group
        PIECE = 4  # k-tiles per probs store

        for qc in range(NQC):
            qsl = bass.ds(qc * QCS, QCS)
            load_mask(qc)
            load_mask(qc + 1)
            mask_sb = mask_tiles[qc]
            ctxn_tiles = []
            for hh in range(HPC):
                pb = pbp.tile([P, KT, QCS], bf16, name=f"pb{qc}_{hh}", tag="pb", bufs=3)
                caug = ps.tile(
                    [HD + 1, QCS], f32, tag="ctx", bufs=2, name=f"caug{qc}_{hh}"
                )
                for g in range(KT // GK):
                    sc = ps.tile(
                        [P, GK * QCS], f32, tag="sc", bufs=2, name=f"sc{qc}_{hh}_{g}"
                    )
                    for j in range(GK):
                        kt = GK * g + j
                        nc.tensor.matmul(
                            sc[:, bass.ds(j * QCS, QCS)],
                            lhsT=kh_view(hh)[:, bass.ds(kt * P, P)],
                            rhs=qh_view(hh)[:, qsl],
                            start=True,
                            stop=False,
                        )
                    for j in range(GK):
                        kt = GK * g + j
                        nc.tensor.matmul(
                            sc[:, bass.ds(j * QCS, QCS)],
                            lhsT=iden_sb,
                            rhs=mask_sb[:, kt, :],
                            start=False,
                            stop=True,
                        )
                    nc.scalar.activation(
                        out=pb[:, bass.ds(GK * g, GK), :],
                        in_=sc,
                        func=AF.Exp,
                        scale=0.125,
                    )
                    for j in range(GK):
                        kt = GK * g + j
                        nc.tensor.matmul(
                            caug,
                            lhsT=vaug[:, kt, hh, :],
                            rhs=pb[:, kt, :],
                            start=(kt == 0),
                            stop=(kt == KT - 1),
                        )
                # softmax denominator -> 1/x as exp(-ln(x)) on ACT (the DVE
                # reciprocal ops hit a codegen bug in this toolchain, and
                # exp/ln/copy/identity share one ACT table set so no reload),
                # then broadcast to 128 partitions with a K=1 PE matmul.
                rs_sb = smp.tile([1, QCS], f32, name=f"rs{qc}_{hh}", tag="rs")
                nc.scalar.activation(out=rs_sb, in_=caug[HD : HD + 1, :], func=AF.Ln)
                recip = smp.tile([1, QCS], f32, name=f"rc{qc}_{hh}", tag="rc")
                nc.scalar.activation(out=recip, in_=rs_sb, func=AF.Exp, scale=-1.0)
                bps = ps.tile([P, QCS], f32, tag="aux", bufs=2, name=f"bps{qc}_{hh}")
                nc.tensor.matmul(bps, lhsT=onesf_sb, rhs=recip, start=True, stop=True)
                aux_sb = smp.tile([P, QCS], f32, name=f"aux{qc}_{hh}", tag="aux_sb")
                nc.scalar.activation(out=aux_sb, in_=bps, func=AF.Copy)

                # normalized ctx (bf16) first: it gates the hidden projection,
                # so don't queue it behind the 16 probs-normalize TTs on DVE
                ctxn = smp.tile(
                    [HD, QCS], bf16, name=f"ctxn{qc}_{hh}", tag=f"ctxn{hh}"
                )
                nc.vector.tensor_mul(ctxn, caug[0:HD, :], aux_sb[0:HD, :])
                ctxn_tiles.append(ctxn)

                # normalized probs -> fp32 -> DRAM (stored [k, q]), 4-kt pieces
                for pc in range(KT // PIECE):
                    pf = pfp.tile(
                        [P, PIECE, QCS], f32, name=f"pf{qc}_{hh}_{pc}", tag="pf",
                        bufs=4,
                    )
                    for j in range(PIECE):
                        kt = pc * PIECE + j
                        nc.vector.tensor_mul(pf[:, j, :], pb[:, kt, :], aux_sb)
                    nc.sync.dma_start(
                        out=probs[
                            hh, bass.ds(pc * PIECE * P, PIECE * P), qsl
                        ].rearrange("(kt p) q -> p kt q", p=P),
                        in_=pf,
                    )


            # partial hidden for this q-chunk: sum over this core's 4 heads
            hid_sb = smp.tile(
                [P, QCS // P, H], f32, name=f"hid{qc}", tag="hid", bufs=1
            )
            for st in range(QCS // P):
                hps = ps.tile([P, H], f32, tag="aux", bufs=2, name=f"hps{qc}_{st}")
                for hh in range(HPC):
                    nc.tensor.matmul(
                        hps,
                        lhsT=ctxn_tiles[hh][:, bass.ds(st * P, P)],
                        rhs=WdT_sb[:, hh, :],
                        start=(hh == 0),
                        stop=(hh == HPC - 1),
                    )
                nc.vector.tensor_copy(out=hid_sb[:, st, :], in_=hps)
                # store per s-tile so the chunk's hidden rows drain as computed
                nc.sync.dma_start(
                    out=hid[bass.ds(qc * QCS + st * P, P), :],
                    in_=hid_sb[:, st, :],
                )

    _split_multi_waits(nc)
    return nc


def _get_program():
    if "nc" not in _cached:
        _cached["nc"] = _build_program()
    return _cached["nc"]


def kernel(query, key, value, pad_mask, Wq, bq, Wv, bv, Wd, bd):
    from concourse import bass_utils

    query = np.asarray(query, dtype=np.float32)
    key = np.asarray(key, dtype=np.float32)
    value = np.asarray(value, dtype=np.float32)
    pad_mask = np.asarray(pad_mask)
    Wq = np.asarray(Wq, dtype=np.float32)
    bq = np.asarray(bq, dtype=np.float32)
    Wv = np.asarray(Wv, dtype=np.float32)
    bv = np.asarray(bv, dtype=np.float32)
    Wd = np.asarray(Wd, dtype=np.float32)
    bd = np.asarray(bd, dtype=np.float32)

    # host-side prep (transposes + bf16 casts + per-core weight slices)
    qT_all = np.ascontiguousarray(query.transpose(0, 2, 1)).astype(BF16)   # [B, c, s]
    kT_all = np.ascontiguousarray(key.transpose(0, 2, 1)).astype(BF16)
    vT_all = np.ascontiguousarray(value.transpose(0, 2, 1)).astype(BF16)
    maskT_all = np.ascontiguousarray(
        (pad_mask.transpose(0, 2, 1).astype(np.float32)) * MASK_BIAS
    ).astype(FP8)                                                           # [B, k, q]
    WqT = np.ascontiguousarray(Wq.T).astype(BF16)    # [c, d]
    WvT = np.ascontiguousarray(Wv.T).astype(BF16)
    WdT = np.ascontiguousarray(Wd.T).astype(BF16)    # [d, o]
    iden = np.eye(P, dtype=np.float32).astype(FP8)
    onesf = np.ones((1, P), np.float32)
    ones1 = np.ones((1, P), np.float32).astype(BF16)

    in_maps = []
    for c in range(NCORES):
        b, hg = c // 2, c % 2
        dsl = slice(hg * HPC * HD, (hg + 1) * HPC * HD)
        in_maps.append(
            {
                "qT": qT_all[b],
                "kTT": kT_all[b],
                "vT": vT_all[b],
                "maskT": maskT_all[b],
                "WqTd": np.ascontiguousarray(WqT[:, dsl]),
                "WvTd": np.ascontiguousarray(WvT[:, dsl]),
                "WdTd": np.ascontiguousarray(WdT[dsl, :]),
                "bq4": np.ascontiguousarray(
                    bq[dsl].reshape(HPC, HD, 1)
                ).astype(np.float32),
                "bv4": np.ascontiguousarray(
                    bv[dsl].reshape(1, HPC, HD)
                ).astype(BF16),
                "iden": iden,
                "onesf": onesf,
                "ones1": ones1,
            }
        )

    nc = _get_program()
    import os

    trace = bool(int(os.environ.get("MHA_TRACE", "0")))
    kw = {}
    if trace:
        kw = dict(trace=True, trace_cores=[0], tmpdir=os.environ.get("MHA_TRACE_DIR"))
    res = bass_utils.run_bass_kernel_spmd(
        nc, in_maps, core_ids=list(range(NCORES)), **kw
    )
    _cached["last_results"] = res

    # assemble outputs
    probs_kq = np.empty((B, 2, HPC, S, S), np.float32)
    hidden = np.empty((B, S, H), np.float32)
    for c in range(NCORES):
        b, hg = c // 2, c % 2
        probs_kq[b, hg] = res.results[c]["probs"]
        if hg == 0:
            hidden[b] = res.results[c]["hid"]
        else:
            hidden[b] += res.results[c]["hid"]
    hidden += bd[None, None, :]
    # [B, hg, hh, k, q] -> [B, NH, q, k] as a zero-copy view
    attn_probs = probs_kq.reshape(B, NH, S, S).transpose(0, 1, 3, 2)
    return hidden, attn_probs
